# revision 5
# baseline (speedup 1.0000x reference)
"""Contrastive-loss kernel for 8 Trainium2 NeuronCores.

loss = (1/N) * sum_ij [ same_ij * relu(1 - s_ij) + (1-same_ij) * s_ij * 1[s_ij > 0.3] ]
where s = X @ X.T and same_ij = (t_i == t_j).

Strategy:
  * Host sorts rows by target class (loss is permutation invariant). Same-class
    pairs then form contiguous blocks on the diagonal, so the masked term only
    needs a narrow diagonal band; everything else is the unmasked neg term.
  * sum_ij neg(s) over ALL pairs: neg(s) = relu(s-0.3) + 0.3*1[s>0.3]. Per
    matmul tile, one ScalarE relu(s-0.3) with row-sum accumulator and one
    VectorE is_gt count with row-sum accumulator.
  * Band correction per row-tile: sum over same-pairs of (relu(1-s) - neg(s)),
    computed on a W-wide band around the diagonal with an exact same-mask.
    This also cancels the diagonal's neg(s_ii) from the unmasked pass.
  * Each of the 8 cores owns 1024 rows (data-parallel, no collectives); the
    full X^T lives in SBUF as the moving matmul operand (bf16, fp32 PSUM).
  * Cores emit [128, 4] fp32 per-partition partials; host reduces in float64.
"""

from contextlib import ExitStack

import numpy as np
import ml_dtypes

import concourse.bass as bass
import concourse.mybir as mybir
import concourse.tile as tile
from concourse import bass_utils
from concourse.vector_clock import ScopedClock

N = 8192
D = 512
NCORES = 8
MROWS = N // NCORES        # rows per core
MT = MROWS // 128          # row tiles per core
NT = N // 512              # col tiles
KT = D // 128              # contraction tiles
MARGIN = 0.3

F32 = mybir.dt.float32
BF16 = mybir.dt.bfloat16
ALU = mybir.AluOpType
ACTF = mybir.ActivationFunctionType





def _legalize_sync_waits(nc: bass.Bass) -> None:
    """This walrus build rejects instructions carrying more than one sync wait
    ("Too many sync wait commands" in setupSyncWait). Keep one wait per
    instruction and hoist the rest onto single-wait EventSemaphore
    instructions inserted just before it on the same engine (engines execute
    their stream in order, so semantics are preserved)."""
    for func in nc.m.functions:
        for bb in func.blocks:
            out = []
            changed = False
            for inst in bb.instructions:
                si = inst.sync_info
                if si is not None and si.on_wait and len(si.on_wait) > 1:
                    waits = list(si.on_wait)
                    inst.sync_info = mybir.SyncInfo(
                        on_wait=[waits[-1]], on_update=list(si.on_update or [])
                    )
                    for w in waits[:-1]:
                        ev = mybir.InstEventSemaphore(
                            name=nc.get_next_instruction_name(),
                            ins=[],
                            outs=[],
                            sync_info=mybir.SyncInfo(on_wait=[w], on_update=[]),
                        )
                        ev.engine = inst.engine
                        out.append(ev)
                    changed = True
                out.append(inst)
            if changed:
                bb.instructions = out


def _build(w: int) -> bass.Bass:
    """Build the SPMD program. w = diagonal band width (multiple of 128, <=512)."""
    nc = bass.Bass("TRN2", target_bir_lowering=False, debug=False)
    # activation() lowers a non-Copy float bias to a const AP; register it.
    _c = nc.alloc_sbuf_tensor("const-float32-negmargin", [128, 1], F32)
    nc.gpsimd.memset(_c.ap(), -MARGIN)
    nc.const_aps.aps[(F32, -MARGIN)] = _c.ap()
    nc.all_engine_barrier()

    xt = nc.dram_tensor("xt", [KT, 128, N], BF16, kind="ExternalInput").ap()
    lhs = nc.dram_tensor("lhs", [KT, 128, MROWS], BF16, kind="ExternalInput").ap()
    bandx = nc.dram_tensor("bandx", [KT, 128, MT * w], BF16, kind="ExternalInput").ap()
    tband = nc.dram_tensor("tband", [128, MT * w], F32, kind="ExternalInput").ap()
    trow = nc.dram_tensor("trow", [128, MT], F32, kind="ExternalInput").ap()
    out = nc.dram_tensor("out", [128, 4], F32, kind="ExternalOutput").ap()

    with tile.TileContext(nc) as tc, ExitStack() as ctx:
        resident = ctx.enter_context(tc.tile_pool(name="resident", bufs=1))
        psum_pool = ctx.enter_context(tc.tile_pool(name="psum", bufs=4, space="PSUM"))
        bpsum_pool = ctx.enter_context(tc.tile_pool(name="bpsum", bufs=2, space="PSUM"))
        rs_pool = ctx.enter_context(tc.tile_pool(name="rs", bufs=6))
        cs_pool = ctx.enter_context(tc.tile_pool(name="cs", bufs=6))
        band_pool = ctx.enter_context(tc.tile_pool(name="band", bufs=2))

        xt_t = [resident.tile([128, N], BF16, tag=f"xt{k}", name=f"xt{k}") for k in range(KT)]
        lhs_t = [resident.tile([128, MROWS], BF16, tag=f"lhs{k}", name=f"lhs{k}") for k in range(KT)]
        bandx_t = [resident.tile([128, MT * w], BF16, tag=f"bx{k}", name=f"bx{k}") for k in range(KT)]
        tband_t = resident.tile([128, MT * w], F32, tag="tband")
        trow_t = resident.tile([128, MT], F32, tag="trow")
        rbuf = resident.tile([128, MT * NT], F32, tag="rbuf")
        cbuf = resident.tile([128, MT * NT], F32, tag="cbuf")
        corrbuf = resident.tile([128, MT], F32, tag="corrbuf")
        out_sb = resident.tile([128, 4], F32, tag="out_sb")

        for k in range(KT):
            nc.sync.dma_start(lhs_t[k][:], lhs[k, :, :])
        nc.sync.dma_start(trow_t[:], trow[:, :])
        nc.sync.dma_start(tband_t[:], tband[:, :])
        for k in range(KT):
            nc.sync.dma_start(bandx_t[k][:], bandx[k, :, :])
        for k in range(KT):
            nc.sync.dma_start(xt_t[k][:], xt[k, :, :])

        NGRP = 4
        for m in range(MT):
            ms = slice(m * 128, (m + 1) * 128)

            # ---- unmasked neg pass over the full row-block ----
            for g in range(NT // NGRP):
                ptiles = [psum_pool.tile([128, 512], F32, name="pt") for _ in range(NGRP)]
                for k in range(KT):
                    for j in range(NGRP):
                        n = g * NGRP + j
                        nc.tensor.matmul(
                            ptiles[j][:],
                            lhs_t[k][:, ms],
                            xt_t[k][:, n * 512:(n + 1) * 512],
                            start=(k == 0),
                            stop=(k == KT - 1),
                        )
                for j in range(NGRP):
                    n = g * NGRP + j
                    idx = m * NT + n
                    rt = rs_pool.tile([128, 512], BF16, tag="rt")
                    # rt = relu(s - 0.3); rbuf[:, idx] = row-sum(rt)
                    nc.scalar.activation(
                        rt[:], ptiles[j][:], ACTF.Relu,
                        bias=-MARGIN, scale=1.0,
                        accum_out=rbuf[:, idx:idx + 1],
                    )
                    ct = cs_pool.tile([128, 512], BF16, tag="ct")
                    # ct = 1[rt > 0]; cbuf[:, idx] = row-count(s > 0.3)
                    nc.vector.tensor_scalar(
                        ct[:], rt[:], 0.0, None,
                        op0=ALU.is_gt, op1=ALU.add,
                        accum_out=cbuf[:, idx:idx + 1],
                    )

            # ---- same-pair correction on the diagonal band ----
            bs = slice(m * w, (m + 1) * w)
            bp = bpsum_pool.tile([128, w], F32, tag="bp")
            for k in range(KT):
                nc.tensor.matmul(
                    bp[:], lhs_t[k][:, ms], bandx_t[k][:, bs],
                    start=(k == 0), stop=(k == KT - 1),
                )
            ub = band_pool.tile([128, w], F32, tag="ub")
            nc.scalar.activation(ub[:], bp[:], ACTF.Copy, bias=1.0, scale=-1.0)
            sameb = band_pool.tile([128, w], F32, tag="sameb")
            nc.vector.tensor_scalar(
                sameb[:], tband_t[:, bs], trow_t[:, m:m + 1], None, op0=ALU.is_equal
            )
            negb = band_pool.tile([128, w], F32, tag="negb")
            # negb = 1[u < 1 - 0.3] * s  (u = 1 - s)
            nc.vector.scalar_tensor_tensor(
                negb[:], ub[:], 1.0 - MARGIN, bp[:], op0=ALU.is_lt, op1=ALU.mult
            )
            posb = band_pool.tile([128, w], F32, tag="posb")
            nc.vector.tensor_scalar(posb[:], ub[:], 0.0, None, op0=ALU.max)
            db = band_pool.tile([128, w], F32, tag="db")
            nc.vector.tensor_tensor(db[:], posb[:], negb[:], op=ALU.subtract)
            junk = band_pool.tile([128, w], F32, tag="junk")
            nc.vector.tensor_tensor(junk[:], sameb[:], db[:], op=ALU.mult)
            nc.vector.tensor_reduce(
                corrbuf[:, m:m + 1], junk[:], axis=mybir.AxisListType.X, op=ALU.add
            )

        nc.vector.tensor_reduce(out_sb[:, 0:1], rbuf[:], axis=mybir.AxisListType.X, op=ALU.add)
        nc.vector.tensor_reduce(out_sb[:, 1:2], cbuf[:], axis=mybir.AxisListType.X, op=ALU.add)
        nc.vector.tensor_reduce(out_sb[:, 2:3], corrbuf[:], axis=mybir.AxisListType.X, op=ALU.add)
        nc.vector.memset(out_sb[:, 3:4], 0.0)
        nc.sync.dma_start(out[:, :], out_sb[:])

    _legalize_sync_waits(nc)
    return nc


_cache: dict[int, bass.Bass] = {}


def _get_program(w: int) -> bass.Bass:
    if w not in _cache:
        _cache[w] = _build(w)
    return _cache[w]


def _prep_inputs(inputs: np.ndarray, targets: np.ndarray, w: int):
    """Sort rows by class, build per-core input maps."""
    t = np.asarray(targets).reshape(-1)
    x = np.asarray(inputs, dtype=np.float32)
    order = np.argsort(t, kind="stable")
    xs = x[order]
    ts = t[order].astype(np.float32)

    xt_host = np.ascontiguousarray(xs.T).astype(ml_dtypes.bfloat16)  # [D, N]
    xt_in = xt_host.reshape(KT, 128, N)

    half = (w - 128) // 2
    in_maps = []
    for c in range(NCORES):
        r0 = c * MROWS
        lhs_c = np.ascontiguousarray(xt_in[:, :, r0:r0 + MROWS])
        bandx_c = np.empty((KT, 128, MT * w), dtype=ml_dtypes.bfloat16)
        tband_c = np.empty((128, MT * w), dtype=np.float32)
        for m in range(MT):
            c0 = min(max(r0 + m * 128 - half, 0), N - w)
            bandx_c[:, :, m * w:(m + 1) * w] = xt_in[:, :, c0:c0 + w]
            tband_c[:, m * w:(m + 1) * w] = ts[c0:c0 + w][None, :]
        trow_c = np.ascontiguousarray(
            ts[r0:r0 + MROWS].reshape(MT, 128).T
        ).astype(np.float32)
        in_maps.append({
            "xt": xt_in,
            "lhs": lhs_c,
            "bandx": np.ascontiguousarray(bandx_c),
            "tband": tband_c,
            "trow": trow_c,
        })
    return in_maps


def _band_width(targets: np.ndarray) -> int:
    counts = np.bincount(np.asarray(targets).reshape(-1).astype(np.int64))
    b = int(counts.max()) if counts.size else 1
    # band must cover 128 rows plus (B-1) on each side, rounded to 128
    w = 128 + 2 * (((max(b - 1, 1) + 63) // 64) * 64)
    w = max(w, 256)
    if w > 512:
        raise NotImplementedError(
            f"class block of {b} rows needs band width {w} > 512"
        )
    return w


def kernel(inputs: np.ndarray, targets: np.ndarray) -> np.ndarray:
    w = _band_width(targets)
    nc = _get_program(w)
    in_maps = _prep_inputs(inputs, targets, w)
    res = bass_utils.run_bass_kernel_spmd(nc, in_maps, core_ids=list(range(NCORES)))
    total = np.float64(0.0)
    for c in range(NCORES):
        o = res.results[c]["out"].astype(np.float64)
        total += o[:, 0].sum() + MARGIN * o[:, 1].sum() + o[:, 2].sum()
    return np.asarray(np.float32(total / N))
